# revision 22
# baseline (speedup 1.0000x reference)
"""CMPNN message-passing kernel for 8 Trainium2 NeuronCores.

Strategy (fp32 end-to-end; the network amplifies per-op relative error
~100-300x, so no 16-bit anywhere on the data path):
  - atoms/molecules sharded by molecule (125 mols = 5000 atoms per core,
    padded to 5120 rows/core), bonds sharded by contiguous id range
    (20480 rows/core).  Small weights replicated.
  - message_bond / message_atom tables are AllGathered between the two
    message-passing iterations; gathers (a2b / b2a / b2revb) are
    per-core indirect DMAs against the full (gathered) tables.
  - BatchGRU + output head run fully molecule-local in a transposed
    layout (hidden on partitions) so relu/bias/mean become per-partition
    ops and W_o consumes the GRU states without extra transposes.
"""
import os
import sys
import numpy as np

sys.path.insert(0, "/opt/trn_rl_repo")

# ---- problem constants (hardcoded per contract) ----
A = 40001
NB = 6
BN = 160001
ATOM_FDIM = 133
BOND_FDIM = 147
H = 300
N_MOLS = 1000
A_SIZE = 40
NC_ = 8

S_A = 5120            # atom rows per core (125 mols * 40 + pad)
S_B = 20480           # bond rows per core
APAD = NC_ * S_A      # 40960
BPAD = NC_ * S_B      # 163840
NT_A = S_A // 128     # 40 atom tiles
NT_B = S_B // 128     # 160 bond tiles
MOLS = N_MOLS // NC_  # 125
HK = [(0, 128), (128, 128), (256, 44)]   # K-chunks of H=300
F_ATOM_K = [(0, 128), (128, 5)]
F_BOND_K = [(0, 128), (128, 19)]

_BUILD_CACHE = {}


def _atom_map():
    """global atom id -> padded id. Every core: pos0 = pad, 1..5000 = its mols."""
    ids = np.arange(A, dtype=np.int64)
    mol = np.maximum(ids - 1, 0) // A_SIZE
    core = mol // MOLS
    pos = 1 + (ids - 1) - core * (MOLS * A_SIZE)
    pid = core * S_A + pos
    pid[0] = 0
    return pid.astype(np.int64)


def _build_program():
    import concourse.bass as bass
    import concourse.mybir as mybir
    import concourse.tile as tile
    from concourse import bacc
    from concourse.masks import make_identity

    fp32 = mybir.dt.float32
    i32 = mybir.dt.int32
    AX = mybir.AxisListType
    AF = mybir.ActivationFunctionType
    ALU = mybir.AluOpType

    nc = bacc.Bacc("TRN2", target_bir_lowering=False, debug=False,
                   num_devices=NC_)

    # ---------------- I/O declarations ----------------
    def inp(name, shape, dt=fp32):
        return nc.dram_tensor(name, list(shape), dt, kind="ExternalInput").ap()

    faT_hi = inp("faT_hi", (128, S_A))
    faT_lo = inp("faT_lo", (5, S_A))
    fbT_hi = inp("fbT_hi", (128, S_B))
    fbT_lo = inp("fbT_lo", (19, S_B))
    a2b_s = inp("a2b_s", (S_A, NB), i32)
    b2a_s = inp("b2a_s", (S_B,), i32)
    b2revb_s = inp("b2revb_s", (S_B,), i32)
    wia_d = inp("wia", (ATOM_FDIM, H))
    wib_d = inp("wib", (BOND_FDIM, H))
    wh_d = [inp("wh0", (H, H)), inp("wh1", (H, H))]
    wlr_d = inp("wlr", (3 * H, H))
    wo_d = inp("wo", (2 * H, H))
    wihT_d = {d: inp(f"wihT_{d}", (H + 1, 3 * H)) for d in "fb"}
    whhT_d = {d: inp(f"whhT_{d}", (H, 3 * H)) for d in "fb"}
    bhhn_d = {d: inp(f"bhhn_{d}", (1, H)) for d in "fb"}
    gbias_d = inp("gbias_col", (H, 1))
    bo_d = inp("bo_col", (H, 1))

    outv = nc.dram_tensor("outv", [H, MOLS], fp32, kind="ExternalOutput").ap()

    RG = [list(range(NC_))]

    from contextlib import ExitStack
    stack = ExitStack()

    with tile.TileContext(nc) as tc, stack:
        # ------------- persistent DRAM scratch -------------
        dram = stack.enter_context(tc.tile_pool(name="dram", bufs=1, space="DRAM"))
        d_iaT = dram.tile([H, S_A], fp32, name="d_iaT")
        d_ma = dram.tile([S_A, H], fp32, name="d_ma")
        d_ib = dram.tile([S_B, H], fp32, name="d_ib")
        d_mb = [dram.tile([S_B, H], fp32, name=f"d_mb{k}") for k in range(2)]
        d_bfull = [dram.tile([BPAD, H], fp32, name=f"d_bfull{k}",
                             addr_space="Shared")
                   for k in range(3)]   # 0: input_bond, 1/2: after iters
        d_afull = [dram.tile([APAD, H], fp32, name=f"d_afull{k}",
                             addr_space="Shared")
                   for k in range(2)]
        d_outT = dram.tile([2 * H, MOLS * A_SIZE], fp32, name="d_outT")
        d_agg2T = dram.tile([H, S_A], fp32, name="d_agg2T")

        # ------------- constant weights in SBUF -------------
        cst = stack.enter_context(tc.tile_pool(name="cst", bufs=1))

        def load_w(dst_shapes, src, name):
            tiles = []
            r = 0
            for i, (n, w) in enumerate(dst_shapes):
                t = cst.tile([n, w], fp32, name=f"{name}{i}")
                nc.sync.dma_start(out=t[:, :], in_=src[r:r + n, :])
                tiles.append(t)
                r += n
            return tiles

        wia = load_w([(128, H), (5, H)], wia_d, "wia")
        wib = load_w([(128, H), (19, H)], wib_d, "wib")
        wh = [load_w([(n, H) for _, n in HK], wh_d[k], f"wh{k}")
              for k in range(2)]
        # W_lr: rows grouped [agg | message_atom | input_atom], HK chunks each
        wlr = []
        for b in range(3):
            for (o, n) in HK:
                t = cst.tile([n, H], fp32, name=f"wlr{b}_{o}")
                nc.sync.dma_start(out=t[:, :], in_=wlr_d[b * H + o:b * H + o + n, :])
                wlr.append(t)
        # W_o: 6 K-chunks of 600 (two 300-blocks x HK)
        wo = []
        for b in range(2):
            for (o, n) in HK:
                t = cst.tile([n, H], fp32, name=f"wo{b}_{o}")
                nc.sync.dma_start(out=t[:, :], in_=wo_d[b * H + o:b * H + o + n, :])
                wo.append(t)
        gbias = load_w([(n, 1) for _, n in HK], gbias_d, "gbias")
        bo = load_w([(n, 1) for _, n in HK], bo_d, "bo")
        ident = cst.tile([128, 128], fp32, name="ident")
        make_identity(nc, ident[:, :])
        ones1 = cst.tile([1, 128], fp32, name="ones1")
        nc.vector.memset(ones1[:, :], 1.0)

        # ------------- P1: input_atom = relu(f_atoms @ Wi_atom) -------------
        with tc.tile_pool(name="p1", bufs=3) as p1, \
             tc.tile_pool(name="p1ps", bufs=2, space="PSUM") as p1ps:
            for t in range(NT_A):
                cs = slice(t * 128, (t + 1) * 128)
                lhi = p1.tile([128, 128], fp32, name="lhi")
                llo = p1.tile([5, 128], fp32, name="llo")
                nc.sync.dma_start(out=lhi[:, :], in_=faT_hi[:, cs])
                nc.sync.dma_start(out=llo[:, :], in_=faT_lo[:, cs])
                ps = p1ps.tile([128, H], fp32, name="ps")
                nc.tensor.matmul(ps[:, :], lhi[:, :], wia[0][:, :],
                                 start=True, stop=False)
                nc.tensor.matmul(ps[:, :], llo[:, :], wia[1][:, :],
                                 start=False, stop=True)
                ia_t = p1.tile([128, H], fp32, name="ia_t")
                nc.scalar.activation(ia_t[:, :], ps[:, :], AF.Relu)
                nc.sync.dma_start(out=d_ma[t * 128:(t + 1) * 128, :],
                                  in_=ia_t[:, :])
                # transpose -> d_iaT
                tp = p1ps.tile([128, 384], fp32, name="tp")
                cp = p1.tile([128, 384], fp32, name="cp")
                for c, (o, n) in enumerate(HK):
                    nc.tensor.transpose(tp[0:n, c * 128:c * 128 + 128],
                                        ia_t[:, o:o + n], ident[:, :])
                    nc.vector.tensor_copy(cp[0:n, c * 128:c * 128 + 128],
                                          tp[0:n, c * 128:c * 128 + 128])
                    nc.sync.dma_start(out=d_iaT[o:o + n, cs],
                                      in_=cp[0:n, c * 128:c * 128 + 128])

        # ------------- P2: input_bond = relu(f_bonds @ Wi_bond) -------------
        with tc.tile_pool(name="p2", bufs=3) as p2, \
             tc.tile_pool(name="p2ps", bufs=2, space="PSUM") as p2ps:
            for t in range(NT_B):
                cs = slice(t * 128, (t + 1) * 128)
                lhi = p2.tile([128, 128], fp32, name="lhi")
                llo = p2.tile([19, 128], fp32, name="llo")
                nc.sync.dma_start(out=lhi[:, :], in_=fbT_hi[:, cs])
                nc.sync.dma_start(out=llo[:, :], in_=fbT_lo[:, cs])
                ps = p2ps.tile([128, H], fp32, name="ps")
                nc.tensor.matmul(ps[:, :], lhi[:, :], wib[0][:, :],
                                 start=True, stop=False)
                nc.tensor.matmul(ps[:, :], llo[:, :], wib[1][:, :],
                                 start=False, stop=True)
                ib_t = p2.tile([128, H], fp32, name="ib_t")
                nc.scalar.activation(ib_t[:, :], ps[:, :], AF.Relu)
                nc.sync.dma_start(out=d_ib[cs, :], in_=ib_t[:, :])

        nc.gpsimd.collective_compute(
            "AllGather", ALU.bypass, replica_groups=RG,
            ins=[d_ib[:, :]], outs=[d_bfull[0][:, :]])

        # ------------- message passing iterations -------------
        def atom_gather_agg(pool, g, src_full, want_ma=True):
            """gather nei for atom group g (2 tiles), return (agg[128,600], ma[128,600])."""
            r0 = g * 256
            idx = pool.tile([128, 12], i32, name="idx")
            nc.sync.dma_start(out=idx[:, 0:6], in_=a2b_s[r0:r0 + 128, :])
            nc.sync.dma_start(out=idx[:, 6:12], in_=a2b_s[r0 + 128:r0 + 256, :])
            nei = pool.tile([128, 12 * H], fp32, name="nei")
            nc.gpsimd.indirect_dma_start(
                out=nei[:, :].rearrange("p (j h) -> p j h", h=H),
                out_offset=None,
                in_=src_full[:, :],
                in_offset=bass.IndirectOffsetOnAxis(ap=idx[:, :], axis=0))
            nv = nei[:, :].rearrange("p (t j h) -> p t h j", t=2, j=NB, h=H)
            s = pool.tile([128, 2 * H], fp32, name="s")
            m = pool.tile([128, 2 * H], fp32, name="m")
            nc.vector.reduce_sum(s[:, :].rearrange("p (t h) -> p t h", h=H),
                                 nv, axis=AX.X)
            nc.vector.reduce_max(m[:, :].rearrange("p (t h) -> p t h", h=H),
                                 nv, axis=AX.X)
            agg = pool.tile([128, 2 * H], fp32, name="agg")
            nc.vector.tensor_mul(agg[:, :], s[:, :], m[:, :])
            if not want_ma:
                return agg, None
            ma_t = pool.tile([128, 2 * H], fp32, name="ma_t")
            nc.sync.dma_start(out=ma_t[:, 0:H], in_=d_ma[r0:r0 + 128, :])
            nc.sync.dma_start(out=ma_t[:, H:2 * H], in_=d_ma[r0 + 128:r0 + 256, :])
            return agg, ma_t

        for k in range(2):
            src_full = d_bfull[k]
            # ---- atom side: message_atom += sum*max of neighbor bonds ----
            with tc.tile_pool(name=f"at{k}", bufs=2) as ap_:
                for g in range(NT_A // 2):
                    agg, ma_t = atom_gather_agg(ap_, g, src_full)
                    nc.vector.tensor_add(ma_t[:, :], ma_t[:, :], agg[:, :])
                    r0 = g * 256
                    nc.sync.dma_start(out=d_ma[r0:r0 + 128, :], in_=ma_t[:, 0:H])
                    nc.sync.dma_start(out=d_ma[r0 + 128:r0 + 256, :],
                                      in_=ma_t[:, H:2 * H])
            nc.gpsimd.collective_compute(
                "AllGather", ALU.bypass, replica_groups=RG,
                ins=[d_ma[:, :]], outs=[d_afull[k][:, :]])

            # ---- bond side ----
            with tc.tile_pool(name=f"bd{k}", bufs=2) as bp, \
                 tc.tile_pool(name=f"bdps{k}", bufs=2, space="PSUM") as bps:
                for g in range(NT_B // 4):
                    r0 = g * 512
                    idxa = bp.tile([128, 4], i32, name="idxa")
                    idxr = bp.tile([128, 4], i32, name="idxr")
                    for j in range(4):
                        rs = slice(r0 + j * 128, r0 + (j + 1) * 128)
                        nc.sync.dma_start(out=idxa[:, j:j + 1], in_=b2a_s[rs, None])
                        nc.sync.dma_start(out=idxr[:, j:j + 1],
                                          in_=b2revb_s[rs, None])
                    ga = bp.tile([128, 4 * H], fp32, name="ga")
                    gr = bp.tile([128, 4 * H], fp32, name="gr")
                    nc.gpsimd.indirect_dma_start(
                        out=ga[:, :].rearrange("p (j h) -> p j h", h=H),
                        out_offset=None, in_=d_afull[k][:, :],
                        in_offset=bass.IndirectOffsetOnAxis(ap=idxa[:, :], axis=0))
                    nc.gpsimd.indirect_dma_start(
                        out=gr[:, :].rearrange("p (j h) -> p j h", h=H),
                        out_offset=None, in_=src_full[:, :],
                        in_offset=bass.IndirectOffsetOnAxis(ap=idxr[:, :], axis=0))
                    mbv = bp.tile([128, 4 * H], fp32, name="mbv")
                    nc.vector.tensor_sub(mbv[:, :], ga[:, :], gr[:, :])
                    for j in range(4):
                        rs = slice(r0 + j * 128, r0 + (j + 1) * 128)
                        tp = bps.tile([128, 384], fp32, name="tp")
                        cp = bp.tile([128, 384], fp32, name="cp")
                        for c, (o, n) in enumerate(HK):
                            nc.tensor.transpose(tp[0:n, c * 128:c * 128 + 128],
                                                mbv[:, j * H + o:j * H + o + n],
                                                ident[:, :])
                            nc.vector.tensor_copy(cp[0:n, c * 128:c * 128 + 128],
                                                  tp[0:n, c * 128:c * 128 + 128])
                        po = bps.tile([128, H], fp32, name="po")
                        for c, (o, n) in enumerate(HK):
                            nc.tensor.matmul(po[:, :],
                                             cp[0:n, c * 128:c * 128 + 128],
                                             wh[k][c][:, :],
                                             start=(c == 0), stop=(c == 2))
                        res = bp.tile([128, H], fp32, name="res")
                        nc.sync.dma_start(out=res[:, :], in_=d_ib[rs, :])
                        nc.vector.tensor_add(res[:, :], res[:, :], po[:, :])
                        nc.scalar.activation(res[:, :], res[:, :], AF.Relu)
                        nc.sync.dma_start(out=d_mb[k][rs, :], in_=res[:, :])
            nc.gpsimd.collective_compute(
                "AllGather", ALU.bypass, replica_groups=RG,
                ins=[d_mb[k][:, :]], outs=[d_bfull[k + 1][:, :]])

        # ------------- final aggregation + W_lr -------------
        # persistent transposed tensors
        big = stack.enter_context(tc.tile_pool(name="big", bufs=1))
        msgT = [big.tile([n, S_A], fp32, name=f"msgT{c}")
                for c, (o, n) in enumerate(HK)]
        h0T = [big.tile([n, MOLS], fp32, name=f"h0T{c}")
               for c, (o, n) in enumerate(HK)]

        with tc.tile_pool(name="fin", bufs=2) as fp_, \
             tc.tile_pool(name="finps", bufs=2, space="PSUM") as fps:
            for g in range(NT_A // 2):
                agg, ma_t = atom_gather_agg(fp_, g, d_bfull[2])
                for j in range(2):
                    t = g * 2 + j
                    cs = slice(t * 128, (t + 1) * 128)
                    # transposes of agg & ma slices; load iaT slices
                    cpa = fp_.tile([128, 384], fp32, name="cpa")
                    cpm = fp_.tile([128, 384], fp32, name="cpm")
                    iat = fp_.tile([128, 384], fp32, name="iat")
                    tpa = fps.tile([128, 384], fp32, name="tpa")
                    tpm = fps.tile([128, 384], fp32, name="tpm")
                    for c, (o, n) in enumerate(HK):
                        ks = slice(c * 128, c * 128 + 128)
                        nc.tensor.transpose(tpa[0:n, ks],
                                            agg[:, j * H + o:j * H + o + n],
                                            ident[:, :])
                        nc.vector.tensor_copy(cpa[0:n, ks], tpa[0:n, ks])
                        nc.tensor.transpose(tpm[0:n, ks],
                                            ma_t[:, j * H + o:j * H + o + n],
                                            ident[:, :])
                        nc.vector.tensor_copy(cpm[0:n, ks], tpm[0:n, ks])
                        nc.sync.dma_start(out=iat[0:n, ks], in_=d_iaT[o:o + n, cs])
                    po = fps.tile([128, H], fp32, name="po")
                    for b, part in enumerate((cpa, cpm, iat)):
                        for c, (o, n) in enumerate(HK):
                            nc.tensor.matmul(po[:, :],
                                             part[0:n, c * 128:c * 128 + 128],
                                             wlr[b * 3 + c][:, :],
                                             start=(b == 0 and c == 0),
                                             stop=(b == 2 and c == 2))
                    a2 = fp_.tile([128, H], fp32, name="a2")
                    nc.vector.tensor_copy(a2[:, :], po[:, :])
                    tp2 = fps.tile([128, 384], fp32, name="tp2")
                    cp2 = fp_.tile([128, 384], fp32, name="cp2")
                    for c, (o, n) in enumerate(HK):
                        ks = slice(c * 128, c * 128 + 128)
                        nc.tensor.transpose(tp2[0:n, ks], a2[:, o:o + n],
                                            ident[:, :])
                        nc.vector.tensor_copy(cp2[0:n, ks], tp2[0:n, ks])
                        nc.sync.dma_start(out=d_agg2T[o:o + n, cs],
                                          in_=cp2[0:n, ks])
                        # messageT = relu(agg2T + gru_bias) incrementally
                        nc.scalar.activation(msgT[c][0:n, cs], cp2[0:n, ks],
                                             AF.Relu, bias=gbias[c][:, 0:1])

        # h0T = per-molecule max over the 40 atoms (pre-bias agg2)
        with tc.tile_pool(name="h0p", bufs=1) as h0p:
            for c, (o, n) in enumerate(HK):
                a2T_c = h0p.tile([128, S_A], fp32, name="a2T_c", bufs=2)
                nc.sync.dma_start(out=a2T_c[0:n, :], in_=d_agg2T[o:o + n, :])
                nc.vector.reduce_max(
                    h0T[c][:, :],
                    a2T_c[0:n, 1:1 + MOLS * A_SIZE].rearrange(
                        "p (m a) -> p m a", a=A_SIZE),
                    axis=AX.X)

        # ------------- BatchGRU (both directions) -------------
        gru_cst = stack.enter_context(tc.tile_pool(name="gru_cst", bufs=1))

        def load_gw(dst_shapes, src, name):
            tiles = []
            r = 0
            for i, (n, w) in enumerate(dst_shapes):
                t = gru_cst.tile([n, w], fp32, name=f"{name}{i}")
                nc.sync.dma_start(out=t[:, :], in_=src[r:r + n, :])
                tiles.append(t)
                r += n
            return tiles

        wih = {}
        whh = {}
        bhn = {}
        bx = {}
        for d in "fb":
            wih[d] = load_gw([(n, 3 * H) for _, n in HK], wihT_d[d], f"wih{d}")
            whh[d] = load_gw([(n, 3 * H) for _, n in HK], whhT_d[d], f"whh{d}")
            t = gru_cst.tile([1, H], fp32, name=f"bhn{d}")
            nc.sync.dma_start(out=t[:, :], in_=bhhn_d[d][:, :])
            bhn[d] = t
            t2 = gru_cst.tile([1, 3 * H], fp32, name=f"bx{d}")
            nc.sync.dma_start(out=t2[:, :], in_=wihT_d[d][H:H + 1, :])
            bx[d] = t2

        with tc.tile_pool(name="gru", bufs=2) as gp, \
             tc.tile_pool(name="grups", bufs=1, space="PSUM") as gps, \
             tc.tile_pool(name="grupst", bufs=2, space="PSUM") as gpst:
            # h0 canonical (shared by both dirs)
            ph0 = gpst.tile([MOLS, 384], fp32, name="ph0", bufs=1)
            for c, (o, n) in enumerate(HK):
                nc.tensor.transpose(ph0[:, o:o + n], h0T[c][:, :],
                                    ident[0:n, 0:n])
            h0c = gp.tile([MOLS, H], fp32, name="h0c", bufs=1)
            nc.vector.tensor_copy(h0c[:, :], ph0[:, 0:H])

            hst = {}
            for d in "fb":
                hst[d] = big.tile([MOLS, H], fp32, name=f"h_{d}")
                nc.vector.tensor_copy(hst[d][:, :], h0c[:, :])

            def store_outT(d, seqpos, src_tiles):
                """src_tiles[c] = [n,MOLS] transposed state; write to d_outT."""
                base = 0 if d == "f" else H
                for c, (o, n) in enumerate(HK):
                    nc.sync.dma_start(
                        out=d_outT[base + o:base + o + n,
                                   seqpos * MOLS:(seqpos + 1) * MOLS],
                        in_=src_tiles[c])

            for step in range(A_SIZE + 1):
                for d in "fb":
                    h = hst[d]
                    if step > 0:
                        # transpose h_{step-1} (for gh matmul + output store)
                        tph = gpst.tile([128, 384], fp32, name="tph")
                        cph = gp.tile([128, 384], fp32, name="cph")
                        for c, (o, n) in enumerate(HK):
                            ks = slice(c * 128, c * 128 + MOLS)
                            nc.tensor.transpose(tph[0:n, ks], h[:, o:o + n],
                                                ident[0:MOLS, 0:MOLS])
                            nc.vector.tensor_copy(cph[0:n, ks], tph[0:n, ks])
                        prev = step - 1
                        seqpos = prev if d == "f" else A_SIZE - 1 - prev
                        store_outT(d, seqpos,
                                   [cph[0:n, c * 128:c * 128 + MOLS]
                                    for c, (o, n) in enumerate(HK)])
                        hT = [cph[0:n, c * 128:c * 128 + MOLS]
                              for c, (o, n) in enumerate(HK)]
                    else:
                        hT = [h0T[c][:, :] for c, (o, n) in enumerate(HK)]
                    if step == A_SIZE:
                        continue
                    t = step
                    seqpos = t if d == "f" else A_SIZE - 1 - t
                    col0 = 1 + seqpos
                    pr = gps.tile([MOLS, H], fp32, name=f"pr{d}", tag="pr")
                    pz = gps.tile([MOLS, H], fp32, name=f"pz{d}", tag="pz")
                    pnx = gps.tile([MOLS, H], fp32, name=f"pnx{d}", tag="pnx")
                    pnh = gps.tile([MOLS, H], fp32, name=f"pnh{d}", tag="pnh")
                    # xp side
                    for c in range(3):
                        n = HK[c][1]
                        lhs = msgT[c][0:n, col0:col0 + A_SIZE * MOLS:A_SIZE]
                        nc.tensor.matmul(pr[:, :], lhs, wih[d][c][0:n, 0:H],
                                         start=(c == 0), stop=False)
                        nc.tensor.matmul(pz[:, :], lhs, wih[d][c][0:n, H:2 * H],
                                         start=(c == 0), stop=False)
                        nc.tensor.matmul(pnx[:, :], lhs,
                                         wih[d][c][0:n, 2 * H:3 * H],
                                         start=(c == 0), stop=False)
                    # gate biases via K=1 ones matmul
                    nc.tensor.matmul(pr[:, :], ones1[0:1, 0:MOLS],
                                     bx[d][:, 0:H], start=False, stop=False)
                    nc.tensor.matmul(pz[:, :], ones1[0:1, 0:MOLS],
                                     bx[d][:, H:2 * H], start=False, stop=False)
                    nc.tensor.matmul(pnx[:, :], ones1[0:1, 0:MOLS],
                                     bx[d][:, 2 * H:3 * H], start=False,
                                     stop=True)
                    # gh side
                    for c, (o, n) in enumerate(HK):
                        nc.tensor.matmul(pr[:, :], hT[c], whh[d][c][:, 0:H],
                                         start=False, stop=(c == 2))
                        nc.tensor.matmul(pz[:, :], hT[c], whh[d][c][:, H:2 * H],
                                         start=False, stop=(c == 2))
                        nc.tensor.matmul(pnh[:, :], hT[c],
                                         whh[d][c][:, 2 * H:3 * H],
                                         start=(c == 0), stop=False)
                    nc.tensor.matmul(pnh[:, :], ones1[0:1, 0:MOLS],
                                     bhn[d][:, :], start=False, stop=True)
                    r = gp.tile([MOLS, H], fp32, name="r")
                    z = gp.tile([MOLS, H], fp32, name="z")
                    nc.scalar.activation(r[:, :], pr[:, :], AF.Sigmoid)
                    nc.scalar.activation(z[:, :], pz[:, :], AF.Sigmoid)
                    t1 = gp.tile([MOLS, H], fp32, name="t1")
                    nc.vector.tensor_mul(t1[:, :], r[:, :], pnh[:, :])
                    t2 = gp.tile([MOLS, H], fp32, name="t2")
                    nc.vector.tensor_add(t2[:, :], t1[:, :], pnx[:, :])
                    nn = gp.tile([MOLS, H], fp32, name="nn")
                    nc.scalar.activation(nn[:, :], t2[:, :], AF.Tanh)
                    t3 = gp.tile([MOLS, H], fp32, name="t3")
                    nc.vector.tensor_sub(t3[:, :], h[:, :], nn[:, :])
                    t4 = gp.tile([MOLS, H], fp32, name="t4")
                    nc.vector.tensor_mul(t4[:, :], z[:, :], t3[:, :])
                    nc.vector.tensor_add(h[:, :], nn[:, :], t4[:, :])

        # ------------- W_o head + per-molecule mean -------------
        KO = [(b * H + o, n) for b in range(2) for (o, n) in HK]
        with tc.tile_pool(name="wop", bufs=2) as wp, \
             tc.tile_pool(name="wops", bufs=2, space="PSUM") as wps:
            msum = [big.tile([n, MOLS], fp32, name=f"msum{c}")
                    for c, (o, n) in enumerate(HK)]
            NTO = A_SIZE // 4   # 10 col-tiles of 500 (4 timesteps each)
            for nt in range(NTO):
                cs = slice(nt * 4 * MOLS, (nt + 1) * 4 * MOLS)
                ots = []
                for kk, (o, n) in enumerate(KO):
                    ot = wp.tile([128, 4 * MOLS], fp32, name=f"ot{kk}",
                                 tag=f"ot{kk}")
                    nc.sync.dma_start(out=ot[0:n, :], in_=d_outT[o:o + n, cs])
                    ots.append(ot)
                for c, (o, n) in enumerate(HK):
                    po = wps.tile([128, 4 * MOLS], fp32, name="po",
                                  tag=f"po{c}")
                    for kk, (ko, kn) in enumerate(KO):
                        nc.tensor.matmul(po[0:n, :], wo[kk][:, o:o + n],
                                         ots[kk][0:kn, :],
                                         start=(kk == 0), stop=(kk == 5))
                    ah = wp.tile([128, 4 * MOLS], fp32, name="ah", tag=f"ah{c}")
                    nc.scalar.activation(ah[0:n, :], po[0:n, :], AF.Relu,
                                         bias=bo[c][:, 0:1])
                    part = wp.tile([128, MOLS], fp32, name="prt", tag=f"prt{c}")
                    nc.vector.reduce_sum(
                        part[0:n, :],
                        ah[0:n, :].rearrange("p (t m) -> p m t", m=MOLS),
                        axis=AX.X)
                    if nt == 0:
                        nc.vector.tensor_copy(msum[c][:, :], part[0:n, :])
                    else:
                        nc.vector.tensor_add(msum[c][:, :], msum[c][:, :],
                                             part[0:n, :])
            for c, (o, n) in enumerate(HK):
                nc.vector.tensor_scalar_mul(msum[c][:, :], msum[c][:, :],
                                            1.0 / A_SIZE)
                nc.sync.dma_start(out=outv[o:o + n, :], in_=msum[c][:, :])

    nc.compile()
    return nc


def _prepare_inputs(inputs):
    """Shard + stage per-core input arrays (host-side, free)."""
    f_atoms = np.asarray(inputs["f_atoms"], np.float32)
    f_bonds = np.asarray(inputs["f_bonds"], np.float32)
    a2b = np.asarray(inputs["a2b"], np.int32)
    b2a = np.asarray(inputs["b2a"], np.int32)
    b2revb = np.asarray(inputs["b2revb"], np.int32)

    amap = _atom_map()
    fa_p = np.zeros((APAD, ATOM_FDIM), np.float32)
    fa_p[amap] = f_atoms
    a2b_p = np.zeros((APAD, NB), np.int32)
    a2b_p[amap] = a2b
    fb_p = np.zeros((BPAD, BOND_FDIM), np.float32)
    fb_p[:BN] = f_bonds
    b2a_p = np.zeros(BPAD, np.int32)
    b2a_p[:BN] = amap[b2a].astype(np.int32)
    b2revb_p = np.zeros(BPAD, np.int32)
    b2revb_p[:BN] = b2revb

    def w(name):
        return np.ascontiguousarray(np.asarray(inputs[name], np.float32))

    shared = {
        "wia": w("Wi_atom"), "wib": w("Wi_bond"),
        "wh0": w("Wh0"), "wh1": w("Wh1"),
        "wlr": w("W_lr"), "wo": w("W_o"),
        "gbias_col": w("gru_bias").reshape(H, 1),
        "bo_col": w("b_o").reshape(H, 1),
    }
    for d, suf in (("f", "_f"), ("b", "_b")):
        gWih = w("gWih" + suf)
        gWhh = w("gWhh" + suf)
        gbih = w("gbih" + suf)
        gbhh = w("gbhh" + suf)
        wihT = np.zeros((H + 1, 3 * H), np.float32)
        wihT[:H] = gWih.T
        wihT[H, 0:2 * H] = (gbih + gbhh)[0:2 * H]
        wihT[H, 2 * H:3 * H] = gbih[2 * H:3 * H]
        shared[f"wihT_{d}"] = wihT
        shared[f"whhT_{d}"] = np.ascontiguousarray(gWhh.T)
        shared[f"bhhn_{d}"] = np.ascontiguousarray(gbhh[2 * H:3 * H].reshape(1, H))

    in_maps = []
    for c in range(NC_):
        asl = slice(c * S_A, (c + 1) * S_A)
        bsl = slice(c * S_B, (c + 1) * S_B)
        faT = np.ascontiguousarray(fa_p[asl].T)
        fbT = np.ascontiguousarray(fb_p[bsl].T)
        m = {
            "faT_hi": faT[0:128], "faT_lo": faT[128:133],
            "fbT_hi": fbT[0:128], "fbT_lo": fbT[128:147],
            "a2b_s": np.ascontiguousarray(a2b_p[asl]),
            "b2a_s": np.ascontiguousarray(b2a_p[bsl]),
            "b2revb_s": np.ascontiguousarray(b2revb_p[bsl]),
        }
        m.update(shared)
        in_maps.append(m)
    return in_maps


def _get_runner():
    """Build (once) a cached jitted SPMD executable over the 8 cores.

    Mirrors bass2jax.run_bass_via_pjrt's multi-core path, but caches the
    jitted function so repeat calls reuse the compiled NEFF executable.
    """
    if "runner" in _BUILD_CACHE:
        return _BUILD_CACHE["runner"]
    import jax
    import concourse.mybir as mybir
    from concourse import bass2jax
    from jax.sharding import Mesh, PartitionSpec, NamedSharding
    from jax.experimental.shard_map import shard_map

    if "nc" not in _BUILD_CACHE:
        _BUILD_CACHE["nc"] = _build_program()
    nc = _BUILD_CACHE["nc"]
    bass2jax.install_neuronx_cc_hook()

    partition_name = (nc.partition_id_tensor.name
                      if nc.partition_id_tensor else None)
    in_names, out_names, out_avals, zero_outs = [], [], [], []
    for alloc in nc.m.functions[0].allocations:
        if not isinstance(alloc, mybir.MemoryLocationSet):
            continue
        name = alloc.memorylocations[0].name
        if alloc.kind == "ExternalInput":
            if name != partition_name:
                in_names.append(name)
        elif alloc.kind == "ExternalOutput":
            out_names.append(name)
            shape = tuple(alloc.tensor_shape)
            dtype = mybir.dt.np(alloc.dtype)
            out_avals.append(jax.core.ShapedArray(shape, dtype))
            zero_outs.append(np.zeros(shape, dtype))
    n_params = len(in_names)
    n_outs = len(out_avals)
    all_names = in_names + out_names
    if partition_name is not None:
        all_names = all_names + [partition_name]

    def _body(*args):
        operands = list(args)
        if partition_name is not None:
            operands.append(bass2jax.partition_id_tensor())
        outs = bass2jax._bass_exec_p.bind(
            *operands,
            out_avals=tuple(out_avals),
            in_names=tuple(all_names),
            out_names=tuple(out_names),
            lowering_input_output_aliases=(),
            sim_require_finite=True,
            sim_require_nnan=True,
            nc=nc,
        )
        return tuple(outs)

    devices = jax.devices()[:NC_]
    mesh = Mesh(np.asarray(devices), ("core",))
    donate = tuple(range(n_params, n_params + n_outs))
    sharded = jax.jit(
        shard_map(_body, mesh=mesh,
                  in_specs=(PartitionSpec("core"),) * (n_params + n_outs),
                  out_specs=(PartitionSpec("core"),) * n_outs,
                  check_rep=False),
        donate_argnums=donate, keep_unused=True)
    sharding = NamedSharding(mesh, PartitionSpec("core"))
    runner = dict(fn=sharded, in_names=in_names, out_names=out_names,
                  out_avals=out_avals, zero_outs=zero_outs,
                  sharding=sharding, mesh=mesh)
    _BUILD_CACHE["runner"] = runner
    return runner


def _concat_inputs(in_maps, runner):
    return [np.concatenate([in_maps[c][nm] for c in range(NC_)], axis=0)
            for nm in runner["in_names"]]


def _run(concat_in, runner):
    zeros = [np.zeros((NC_ * z.shape[0], *z.shape[1:]), z.dtype)
             for z in runner["zero_outs"]]
    out_arrs = runner["fn"](*concat_in, *zeros)
    return out_arrs


def kernel(**inputs) -> np.ndarray:
    assert int(inputs.get("n_mols", N_MOLS)) == N_MOLS
    assert int(inputs.get("a_size", A_SIZE)) == A_SIZE

    runner = _get_runner()
    in_maps = _prepare_inputs(inputs)
    concat_in = _concat_inputs(in_maps, runner)
    out_arrs = _run(concat_in, runner)
    oi = runner["out_names"].index("outv")
    full = np.asarray(out_arrs[oi]).reshape(NC_, H, MOLS)
    out = np.concatenate([full[c].T for c in range(NC_)], axis=0)
    return out.astype(np.float32)


if __name__ == "__main__":
    # build-only smoke test (no device execution)
    nc = _build_program()
    n_inst = sum(len(bb.instructions) for f in nc.m.functions for bb in f.blocks)
    print(f"build OK, {n_inst} instructions")
